# revision 48
# baseline (speedup 1.0000x reference)
"""Trainium2 Bass kernel for nn_Attn_head_89412629168239.

The reference computes:
    seq_fts = x @ W1.T + b1            # [55, 8192]
    coefs   = softmax over a size-1 axis = 1.0 identically
    out     = elu(seq_fts)[:, :, None]

so the kernel computes out = elu(x @ W1.T + b1)[:, :, None], column-parallel
over out_sz across 8 NeuronCores (1024 columns of W1 per core).

This version ships the weights as uint8 (memory-bound problem: HBM bytes are
the floor) and dequantizes on-chip:

  host:  s = absmax(W1)/127;  q = round(W/s) in [-127,127];  u = q + 128
  chip:  moving fp16 value (1024+u)*2^-13 is produced from the raw byte u by
         pure bit ops on uint16 lanes (byte-interleaved image [B|A] per lane):
           evens: (lane & 0x00FF) | 0x3000   -> fp16 bits of the A column
           odds:  (lane >> 8)     | 0x3000   -> fp16 bits of the B column
         Each is one 2-op DVE tensor_scalar running in 4x_2P mode (16-bit,
         single-src, SBUF): the whole dequant is 2 passes at 4 elem/cycle.
  PE:    stationary xs = fp16(x * s * 8192)  =>  psum accumulates
         x @ (s*q).T + 1152*s*rowsum(x)  directly in final units.
         The constant offset is cancelled by a K=2 f32 bias matmul
         ([ones; rowsum(xs)] x [b1; -0.140625]) that also adds b1.
  The two 512-column halves run as 2x column-tiled concurrent matmuls
  (M=55 <= 64): group A in psum bank0 partitions 0-54, group B in bank1
  partitions 64-118, so the PE streams two moving operands per cycle.

Epilogue (identical math to the f32 baseline):
  elu(v) = (max(v,0) - 1) + min(exp(v), 1),  exp on ACT, rest on DVE.
"""

import sys

sys.path.insert(0, "/opt/trn_rl_repo")

import ml_dtypes
import numpy as np

from concourse import bacc, bass, mybir, tile
from concourse.bass_utils import run_bass_kernel_spmd
from concourse.vector_clock import ScopedClock

# If the caller enables tracing (e.g. BASS_TRACE=1), bass_utils imports
# antenv.axon_hooks, which this container's stub antenv package lacks.
# Register a minimal implementation so tracing degrades gracefully.
try:
    import antenv.axon_hooks  # noqa: F401
except ImportError:
    try:
        import types as _types

        import antenv as _antenv

        _hooks_mod = _types.ModuleType("antenv.axon_hooks")
        _hook_box = [None]
        _hooks_mod.set_axon_ntff_profile_hook = (
            lambda h: _hook_box.__setitem__(0, h)
        )
        _hooks_mod.get_axon_ntff_profile_hook = lambda: _hook_box[0]
        sys.modules["antenv.axon_hooks"] = _hooks_mod
        _antenv.axon_hooks = _hooks_mod
    except Exception:
        pass


class _LightTailTC(tile.TileContext):
    """TileContext with a lighter kernel tail (see baseline notes)."""

    def _drain_and_barrier(self, tick_clock, wait_clock):
        nc = self.nc
        drain_inst = nc.sync.drain()
        wait_clock.add_sem_waits(
            drain_inst.ins, ScopedClock({None: tick_clock.global_clock})
        )
        gate = nc.gpsimd.nop(nofuse=True, hint="tail_gate")
        wait_clock.add_sem_waits(
            gate.ins, ScopedClock({None: tick_clock.global_clock})
        )
        assert self.sems is not None
        popped = nc._tile_sem_poison_stack.pop()
        assert popped is self._sem_poison
        nc.clear_and_free_semaphores(list(self.sems.allocated().values()))


N_NODES = 55
IN_CH = 8192
OUT_SZ = 8192
N_CORES = 8
O_SHARD = OUT_SZ // N_CORES  # 1024 output columns per core
P = 128
KT = IN_CH // P  # 64 k-tiles
NH = O_SHARD // 2  # 512: column-group size (A = cols 0:512, B = 512:1024)
# weight-DMA chunk sizes in k-tiles: small first chunk so dequant + matmuls
# start early; small tail chunks so the post-DMA pipeline drain is short;
# mid chunks kept moderate so per-chunk dequant latency stays off the path
CHUNK_KOS = [4, 8, 10, 10, 10, 8, 6, 4, 2, 1, 1]
CKMAX = max(CHUNK_KOS)
assert sum(CHUNK_KOS) == KT

# per k-row byte layout: [interleaved 2*CM | plainA 512-CM | plainB 512-CM].
# DVE converts the interleaved region (2 passes at 4x); ACT numerically
# converts the plain regions (Copy, scale=2^-13, bias=0.125). CM balances
# the two engines' per-k-tile time (DVE ~0.52*CM ns, ACT ~2.67*(512-CM) ns).
# The final two 1-k-tile chunks are fully interleaved (CM=512, no ACT op) so
# the post-stream tail depends only on the DVE passes.
CM = 428
CPLAIN = NH - CM  # 84
N_FULL_TAIL = 2  # trailing chunks converted entirely by DVE

MAGIC16 = 0x3000  # fp16 exponent field 2^-3; mantissa = payload byte
MSCALE = float(2.0**-13)
MOFF = 0.125  # 1024 * 2^-13
CCORR = -1152.0 * MSCALE  # -0.140625: cancels the (1024+128) payload offset

U8 = mybir.dt.uint8
U16 = mybir.dt.uint16
F16 = mybir.dt.float16
BF16 = mybir.dt.bfloat16
F32 = mybir.dt.float32
AF = mybir.ActivationFunctionType
ALU = mybir.AluOpType

_cache: dict = {}


def _build_nc():
    nc = bacc.Bacc(None)
    w8_d = nc.dram_tensor("w8", [P, KT, O_SHARD], U8, kind="ExternalInput")
    xs_d = nc.dram_tensor("xs", [P, KT, N_NODES], F16, kind="ExternalInput")
    bias_d = nc.dram_tensor("bias", [2, O_SHARD], F32, kind="ExternalInput")
    stat_d = nc.dram_tensor("stat", [2, N_NODES], F32, kind="ExternalInput")
    # [group, node, col]: each group's store is DRAM-contiguous (fewer, fatter
    # DMA descriptors than a strided [node, 1024] layout); host reassembles
    out_d = nc.dram_tensor("out", [2, N_NODES, NH], BF16, kind="ExternalOutput")

    with _LightTailTC(nc) as tc:
        with (
            tc.tile_pool(name="w8p", bufs=1) as wpool,
            tc.tile_pool(name="wab", bufs=4) as abpool,
            tc.tile_pool(name="misc", bufs=1) as mpool,
            tc.tile_pool(name="eps", bufs=2) as epool,
            tc.tile_pool(name="psum", bufs=1, space="PSUM") as ppool,
        ):
            xs = mpool.tile([P, KT, N_NODES], F16, name="xs_sb")
            biasb = mpool.tile([2, O_SHARD], F32, name="bias_sb")
            statb = mpool.tile([2, N_NODES], F32, name="stat_sb")
            zb = mpool.tile([128, 1], F32, name="zb_sb")
            outs = mpool.tile([128, NH], BF16, name="outs_sb")
            w8cs = [
                wpool.tile([P, cko, O_SHARD], U8, name=f"w8{c}", tag=f"w8{c}")
                for c, cko in enumerate(CHUNK_KOS)
            ]

            nc.vector.memset(zb[:], 0.0)
            # all input streams on the SP HWDGE ring; issue order = ring
            # order, so chunk0 goes absolutely first (compute is gated on it)
            ko_starts = []
            ko0 = 0
            for cko in CHUNK_KOS:
                ko_starts.append(ko0)
                ko0 += cko
            nc.sync.dma_start(out=statb[:], in_=stat_d[:])
            nc.sync.dma_start(out=biasb[:], in_=bias_d[:])
            nc.sync.dma_start(out=w8cs[0][:], in_=w8_d[:, 0 : CHUNK_KOS[0], :])
            nc.sync.dma_start(
                out=w8cs[1][:],
                in_=w8_d[:, ko_starts[1] : ko_starts[1] + CHUNK_KOS[1], :],
            )
            # xs rides the ACT HWDGE ring, enqueued first: the rings drain
            # FIFO (shared queue rows), so xs transfers alone during the boot
            # window and the weight stream then runs uninterrupted
            nc.scalar.dma_start(out=xs[:], in_=xs_d[:])
            # remaining chunks issue after the entry barrier; the ring stays
            # fed because issues run well ahead of the HBM stream
            for c in range(2, len(CHUNK_KOS)):
                nc.sync.dma_start(
                    out=w8cs[c][:],
                    in_=w8_d[:, ko_starts[c] : ko_starts[c] + CHUNK_KOS[c], :],
                )

            psA = ppool.tile([128, NH], F32, name="psA", tag="psA")
            psB = ppool.tile([128, NH], F32, name="psB", tag="psB")

            # K=2 bias matmuls: add b1 and cancel the payload offset.
            # Only need the tiny stat/bias DMAs, so they run first and keep
            # the accumulation tail free of f32 matmuls.
            nc.tensor.matmul(
                psA[0:N_NODES, :], statb[:, :], biasb[:, 0:NH],
                start=True, stop=False,
            )
            nc.tensor.matmul(
                psB[64 : 64 + N_NODES, :], statb[:, :], biasb[:, NH:O_SHARD],
                start=True, stop=False,
            )

            for c, cko in enumerate(CHUNK_KOS):
                w8c = w8cs[c]
                wa = abpool.tile([P, CKMAX, NH], F16, name=f"wa{c}", tag="wa")
                wb = abpool.tile([P, CKMAX, NH], F16, name=f"wb{c}", tag="wb")
                cm = NH if c >= len(CHUNK_KOS) - N_FULL_TAIL else CM
                src16 = w8c[:, :, 0 : 2 * cm].bitcast(U16)  # lanes [B|A]
                # evens -> group A, odds -> group B; both 4x_2P DVE passes
                nc.vector.tensor_scalar(
                    wa[:, :cko, 0:cm].bitcast(U16), src16,
                    0x00FF, MAGIC16, ALU.bitwise_and, ALU.bitwise_or,
                )
                nc.vector.tensor_scalar(
                    wb[:, :cko, 0:cm].bitcast(U16), src16,
                    8, MAGIC16, ALU.logical_shift_right, ALU.bitwise_or,
                )
                if cm < NH:
                    # numeric tail columns on ACT (idle until the epilogue)
                    nc.scalar.activation(
                        wa[:, :cko, cm:NH],
                        w8c[:, :, 2 * cm : 2 * cm + CPLAIN],
                        AF.Copy, bias=MOFF, scale=MSCALE,
                    )
                    nc.scalar.activation(
                        wb[:, :cko, cm:NH],
                        w8c[:, :, 2 * cm + CPLAIN : O_SHARD],
                        AF.Copy, bias=MOFF, scale=MSCALE,
                    )
                for ki in range(cko):
                    kt = ko_starts[c] + ki
                    last = kt == KT - 1
                    nc.tensor.matmul(
                        psA[0:N_NODES, :], xs[:, kt, :], wa[:, ki, :],
                        start=False, stop=last,
                    )
                    nc.tensor.matmul(
                        psB[64 : 64 + N_NODES, :], xs[:, kt, :], wb[:, ki, :],
                        start=False, stop=last,
                    )

            # elu(v) = (max(v,0) - 1) + min(exp(v), 1); one full-width group
            # per psum bank, store issued as soon as that group's outs ready
            # bf16 intermediates: stt runs in 2x DVE mode; rounding is well
            # inside the error budget (0.42% vs the 2% gate)
            groups = [(psA[0:N_NODES, :], 0, 0, nc.sync),
                      (psB[64 : 64 + N_NODES, :], 64, 1, nc.scalar)]
            for ps, rb, oidx, store_eng in groups:
                g = oidx
                r = epool.tile([128, NH], BF16, name=f"r{g}", tag=f"r{g}")
                e = epool.tile([128, NH], BF16, name=f"e{g}", tag=f"e{g}")
                nc.vector.tensor_scalar(
                    r[rb : rb + N_NODES, :], ps, 0.0, -1.0, ALU.max, ALU.add
                )
                nc.scalar.activation(
                    e[rb : rb + N_NODES, :], ps, AF.Exp,
                    bias=zb[rb : rb + N_NODES, 0:1],
                )
                nc.vector.scalar_tensor_tensor(
                    outs[rb : rb + N_NODES, :],
                    e[rb : rb + N_NODES, :],
                    1.0,
                    r[rb : rb + N_NODES, :],
                    ALU.min,
                    ALU.add,
                )
                store_eng.dma_start(
                    out=out_d[oidx, :, :], in_=outs[rb : rb + N_NODES, :]
                )
    nc.compile()
    _hoist_early_dmas(nc)
    return nc


def _hoist_early_dmas(nc):
    """Move dependency-free early DMA issues (first two weight chunks + the
    tiny bias/stat streams) ahead of the Tile-context preamble.  A HWDGE
    dma_start needs only the boot barrier, and its semaphore update travels
    with the instruction.  Kept to 4 issues: each costs ~0.6us of SP
    sequencer time pre-barrier, and SP is the engine the entry barrier waits
    on last."""
    blocks = nc.m.functions[0].blocks
    main = next(b for b in blocks if b.name == "main")
    tile_bb = max(blocks, key=lambda b: len(b.instructions))
    # priority = desired per-ring issue order: big w80 first so its bytes
    # flow earliest; stat/bias issues are near-free; w81 keeps the ring fed
    prio = ["w80", "stat_sb", "bias_sb", "w81", "xs_sb"]
    moved = []
    for ins in list(tile_bb.instructions):
        if type(ins).__name__ != "InstDMACopy" or len(moved) >= len(prio):
            continue
        out_ap = ins.outs[0]
        memref = getattr(out_ap, "memref", "") or ""
        if not any(memref.startswith(t) for t in prio):
            continue
        si = ins.sync_info
        if si is not None and si.on_wait:
            continue
        tile_bb.instructions.remove(ins)
        moved.append(ins)
    moved.sort(
        key=lambda ins: next(
            i for i, t in enumerate(prio)
            if (getattr(ins.outs[0], "memref", "") or "").startswith(t)
        )
    )
    main.instructions[:0] = moved
    return len(moved)


def _prep_inputs(x, W1, b1):
    """Host-side quantization + layout prep; returns per-core in_maps."""
    x = np.asarray(x, dtype=np.float32)
    W1 = np.asarray(W1, dtype=np.float32)
    b1 = np.asarray(b1, dtype=np.float32)

    s = float(np.abs(W1).max()) / 127.0
    xscale = s * 8192.0

    # xs[p, kt, m] = fp16(x[m, kt*128+p] * xscale)
    xs = (x.T.reshape(KT, P, N_NODES).transpose(1, 0, 2) * xscale).astype(
        np.float16
    )
    # offset correction must use the rounded values actually summed on chip
    rowsum_xs = xs.astype(np.float64).sum(axis=(0, 1)) / xscale  # [55]

    q = np.clip(np.rint(W1 / s), -127, 127).astype(np.int16)
    u_all = (q + 128).astype(np.uint8)  # [8192 rows(n), 8192 cols(k)]

    in_maps = []
    for c in range(N_CORES):
        us = u_all[c * O_SHARD : (c + 1) * O_SHARD]  # [1024, 8192]
        # ut[p, kt, n] = us[n, kt*128+p]
        ut = np.ascontiguousarray(
            us.T.reshape(KT, P, O_SHARD).transpose(1, 0, 2)
        )
        A = ut[:, :, 0:NH]
        B = ut[:, :, NH:O_SHARD]
        # u16 lane i = [B(i) | A(i)]: byte-interleave the first CM columns of
        # each group; the remaining CPLAIN columns are stored plain for ACT.
        # k-tiles of the final N_FULL_TAIL chunks are fully interleaved.
        inter = np.stack([A[:, :, 0:CM], B[:, :, 0:CM]], axis=-1).reshape(
            P, KT, 2 * CM
        )
        img = np.concatenate([inter, A[:, :, CM:NH], B[:, :, CM:NH]], axis=2)
        n_tail_kts = sum(CHUNK_KOS[-N_FULL_TAIL:])
        full = np.stack([A, B], axis=-1).reshape(P, KT, O_SHARD)
        img[:, KT - n_tail_kts :, :] = full[:, KT - n_tail_kts :, :]

        bias_pack = np.stack(
            [b1[c * O_SHARD : (c + 1) * O_SHARD],
             np.full(O_SHARD, CCORR, np.float32)]
        ).astype(np.float32)
        stat_pack = np.stack(
            [np.ones(N_NODES, np.float32), rowsum_xs.astype(np.float32) * xscale]
        ).astype(np.float32)
        in_maps.append(
            {
                "w8": np.ascontiguousarray(img),
                "xs": xs,
                "bias": np.ascontiguousarray(bias_pack),
                "stat": np.ascontiguousarray(stat_pack),
            }
        )
    return in_maps


def _run(inputs: dict, trace: bool = False, tmpdir: str | None = None):
    """Run the kernel; returns (full_output, BassKernelResults)."""
    if "nc" not in _cache:
        _cache["nc"] = _build_nc()
    nc = _cache["nc"]
    in_maps = _prep_inputs(inputs["x"], inputs["W1"], inputs["b1"])
    res = run_bass_kernel_spmd(
        nc, in_maps, core_ids=list(range(N_CORES)), trace=trace, tmpdir=tmpdir
    )
    shards = []
    for i in range(N_CORES):
        o = np.asarray(res.results[i]["out"]).astype(np.float32)  # [2, 55, 512]
        shards.append(np.concatenate([o[0], o[1]], axis=1))  # [55, 1024]
    full = np.concatenate(shards, axis=1)  # [55, 8192] f32
    return full[:, :, None], res


def kernel(**inputs) -> np.ndarray:
    out, _ = _run(inputs, trace=False)
    return out
